# revision 13
# baseline (speedup 1.0000x reference)
"""Trainium2 Bass kernel for CompositionModel (gnn_message_passing).

Model: per-cell MLP over [log1p(X) ++ Z[cell_to_batch]] followed by a
segment-mean over batch labels.

Strategy (v2):
  * Host: log1p(X) precomputed and shipped fp8; cells sorted by segment and
    each segment padded to a multiple of 512 so every 512-cell block is
    single-segment; X blocks shipped twice (side by side) so one fp8
    DoubleRow matmul applies W1x_hi (k-tile 1) and W1x_lo (k-tile 2) -- W1
    is then effectively exact.  Z's contribution (Z @ W1z + b1) is a single
    per-block f32 bias vector applied by the ACT engine during relu1.
  * Device (8 cores, data-parallel over blocks, identical static program):
      L1: 2 fp8 DR matmuls -> PSUM; ACT relu1 (+zb bias, /64 descale) ->
      fp8 h1; L2: per m-half one DR matmul with W2_hi (full block) plus one
      DR matmul with 2*W2_lo on the first half of the columns (first-order
      exact through the segment mean); DVE tensor_scalar does
      bias+relu+cast AND the per-block segment sum via accum_out.
      The third (linear) layer commutes with the segment sum and is applied
      on the host to the 512x256 segment sums instead of 500k cells.
  * Host epilogue: subtract the analytically-known contribution of pad
    cells (xl=0 -> h1 = relu(zb)), combine block sums into segment sums,
    undo the x64 weight scale, apply W3/b3 and divide by true counts.
"""

import numpy as np
import ml_dtypes

import concourse.bacc as bacc
import concourse.mybir as mybir
import concourse.tile as tile
from concourse.bass_utils import run_bass_kernel_spmd

BF16 = ml_dtypes.bfloat16
FP8 = ml_dtypes.float8_e4m3fn

N_CORES = 8
DX = 128
DZ = 32
H = 256
B = 512
BLK = 512          # cells per block == segment pad quantum
SB = 4             # blocks per superblock (one DMA)
NBLK = 132         # blocks per core (fits the fixed reference input)
WSCALE = 64.0      # fp8 pre-scale on W1/W2/b2, divided out at the end

_compiled = {}
_last_in_maps = None


def _build_program(nblk):
    f32 = mybir.dt.float32
    bf16 = mybir.dt.bfloat16
    fp8 = mybir.dt.float8e4
    Alu = mybir.AluOpType
    Act = mybir.ActivationFunctionType
    DR = mybir.MatmulPerfMode.DoubleRow
    nsb = nblk // SB

    nc = bacc.Bacc("TRN2", target_bir_lowering=False, debug=False,
                   num_devices=N_CORES)

    # [super][p][xl0..xl3 | xl0..xl3]  (k-tile stride = SB*BLK)
    xt_d = nc.dram_tensor("xt", [nsb, DX, 2 * SB * BLK], fp8,
                          kind="ExternalInput")
    # [m-half][p, ktile*128] fp8: k1 = 64*W1x_hi, k2 = 64*W1x_lo
    w1_d = nc.dram_tensor("w1", [2, 128, 256], fp8, kind="ExternalInput")
    w2hi_d = nc.dram_tensor("w2hi", [2, 128, 256], fp8, kind="ExternalInput")
    zb_d = nc.dram_tensor("zb", [128, 2 * nblk], f32, kind="ExternalInput")
    # per-(block, m-half) relu2 bias: 64*b2 minus the W2-quantization
    # mean-correction for the block's segment
    b2c_d = nc.dram_tensor("b2c", [128, 2 * nblk], f32, kind="ExternalInput")
    out_d = nc.dram_tensor("out", [128, 2 * nblk], f32, kind="ExternalOutput")

    with tile.TileContext(nc) as tc:
        with tc.tile_pool(name="consts", bufs=1) as cpool, \
             tc.tile_pool(name="xt", bufs=3) as xtpool, \
             tc.tile_pool(name="h1", bufs=3) as h1pool, \
             tc.tile_pool(name="hsc", bufs=2) as hscpool, \
             tc.tile_pool(name="psum", bufs=2, space="PSUM") as psum:

            w1t, w2hit = [], []
            for h in range(2):
                for lst, src in ((w1t, w1_d), (w2hit, w2hi_d)):
                    w = cpool.tile([128, 256], fp8, tag=f"w{len(lst)}_{id(src)}")
                    nc.sync.dma_start(w[:], src[h])
                    lst.append(w[:].rearrange("p (k m) -> p k m", k=2))
            zbt = cpool.tile([128, 2 * nblk], f32, tag="zbt")
            nc.sync.dma_start(zbt[:], zb_d[:])
            b2ct = cpool.tile([128, 2 * nblk], f32, tag="b2ct")
            nc.sync.dma_start(b2ct[:], b2c_d[:])
            zeros = cpool.tile([128, 512], bf16, tag="zeros")
            nc.vector.memset(zeros[:], 0.0)
            outt = cpool.tile([128, 2 * nblk], f32, tag="outt")

            xt_tiles = {}
            ps1_tiles = {}
            h1_tiles = {}
            ps2_tiles = {}

            def load_super(j):
                if j >= nsb:
                    return
                t = xtpool.tile([DX, 2 * SB * BLK], fp8, tag="xt")
                nc.sync.dma_start(t[:], xt_d[j])
                xt_tiles[j] = t

            def emit_l1(i):
                j, b = divmod(i, SB)
                xt = xt_tiles[j]
                xv = xt[:].rearrange("p (k g c) -> p g k c", k=2, g=SB)[:, b]
                ps1 = psum.tile([128, 1024], f32, tag="ps1")
                nc.tensor.matmul(ps1[:, 0:512], w1t[0], xv,
                                 start=True, stop=True, perf_mode=DR)
                nc.tensor.matmul(ps1[:, 512:1024], w1t[1], xv,
                                 start=True, stop=True, perf_mode=DR)
                ps1_tiles[i] = ps1
                if b == SB - 1:
                    xt_tiles.pop(j)

            def emit_relu1(i):
                ps1 = ps1_tiles.pop(i)
                h1 = h1pool.tile([128, 1024], fp8, tag="h1")
                nc.scalar.activation(h1[:, 0:512], ps1[:, 0:512], Act.Relu,
                                     bias=zbt[:, i:i + 1],
                                     scale=1.0 / WSCALE)
                nc.scalar.activation(h1[:, 512:1024], ps1[:, 512:1024],
                                     Act.Relu,
                                     bias=zbt[:, nblk + i:nblk + i + 1],
                                     scale=1.0 / WSCALE)
                h1_tiles[i] = h1

            def emit_l2(i):
                h1 = h1_tiles.pop(i)
                h1v = h1[:].rearrange("p (k c) -> p k c", k=2)
                ps2 = psum.tile([128, 1024], f32, tag="ps2")
                for h in range(2):
                    o = h * 512
                    nc.tensor.matmul(ps2[:, o:o + 512], w2hit[h], h1v,
                                     start=True, stop=True, perf_mode=DR)
                ps2_tiles[i] = ps2

            def emit_relu2(i):
                ps2 = ps2_tiles.pop(i)
                for h in range(2):
                    hsc = hscpool.tile([128, 512], bf16, tag=f"hsc{h}")
                    c = h * nblk + i
                    # out = max(ps2 + b2c, 0); accum_out = sum(out) over cells
                    # DVE is the saturated engine; give every 11th op to ACT
                    if (2 * i + h) % 11 == 10:
                        nc.scalar.activation(
                            hsc[:], ps2[:, h * 512:(h + 1) * 512], Act.Relu,
                            bias=b2ct[:, c:c + 1],
                            accum_out=outt[:, c:c + 1])
                    else:
                        nc.vector.scalar_tensor_tensor(
                            hsc[:], ps2[:, h * 512:(h + 1) * 512],
                            b2ct[:, c:c + 1], zeros[:],
                            op0=Alu.add, op1=Alu.max,
                            accum_out=outt[:, c:c + 1])

            load_super(0)
            load_super(1)
            emit_l1(0)
            emit_relu1(0)
            for i in range(1, nblk):
                if i % SB == 0:
                    load_super(i // SB + 1)
                emit_l1(i)
                emit_relu1(i)
                emit_l2(i - 1)
                emit_relu2(i - 1)
            emit_l2(nblk - 1)
            emit_relu2(nblk - 1)

            nc.sync.dma_start(out_d[:], outt[:])

    nc.compile()
    return nc


def _get_program(nblk):
    if nblk not in _compiled:
        _compiled[nblk] = _build_program(nblk)
    return _compiled[nblk]


def _q8(x):
    return np.asarray(x, dtype=np.float32).astype(FP8)


def kernel(X, Z, W1, b1, W2, b2, W3, b3, cell_to_batch, sample_idx_batch):
    X = np.asarray(X)
    Z = np.asarray(Z, dtype=np.float32)
    W1 = np.asarray(W1, dtype=np.float32)
    b1 = np.asarray(b1, dtype=np.float32)
    W2 = np.asarray(W2, dtype=np.float32)
    b2 = np.asarray(b2, dtype=np.float32)
    W3 = np.asarray(W3, dtype=np.float32)
    b3 = np.asarray(b3, dtype=np.float32)
    c2b = np.asarray(cell_to_batch).astype(np.int64)
    sib = np.asarray(sample_idx_batch).astype(np.int64)

    n = X.shape[0]
    nseg = sib.shape[0]
    seg = sib[c2b]

    # ---- host layout prep -------------------------------------------------
    order = np.argsort(seg, kind="stable")
    seg_sorted = seg[order]
    counts = np.bincount(seg, minlength=nseg).astype(np.int64)
    padded = ((counts + BLK - 1) // BLK) * BLK
    starts = np.concatenate([[0], np.cumsum(padded)])[:nseg]
    total_pad = int(padded.sum())
    nblk = NBLK
    while total_pad > N_CORES * nblk * BLK:  # safety fallback, recompiles
        nblk += SB
    ntot = N_CORES * nblk * BLK
    nb_tot = ntot // BLK
    run_starts = np.concatenate([[0], np.cumsum(counts)])[:nseg]
    ranks = np.arange(n, dtype=np.int64) - run_starts[seg_sorted]
    slots = starts[seg_sorted] + ranks

    xl8 = _q8(np.log1p(np.asarray(X, dtype=np.float32)))
    Xs = np.zeros((ntot, DX), dtype=FP8)
    Xs[slots] = xl8[order]

    # [core, nblk, 128, 512] -> supers [core, nsb, 128, SB*512] doubled
    nsb = nblk // SB
    xtc = Xs.reshape(N_CORES, nsb, SB, BLK, DX).transpose(0, 1, 4, 2, 3)
    xtc = np.ascontiguousarray(xtc).reshape(N_CORES, nsb, DX, SB * BLK)
    xt = np.concatenate([xtc, xtc], axis=3)
    xt = np.ascontiguousarray(xt)

    # per-block segment labels / real-cell counts
    nb_of_seg = (padded // BLK).astype(np.int64)
    blk_label = np.full(nb_tot, -1, dtype=np.int64)
    fill = np.repeat(np.arange(nseg), nb_of_seg)
    blk_label[:fill.shape[0]] = fill
    blk_real = np.zeros(nb_tot, dtype=np.int64)
    blk_real[:fill.shape[0]] = BLK
    last_blk = (starts + padded) // BLK - 1
    blk_real[last_blk] = counts - (nb_of_seg - 1) * BLK

    # ---- weights ----------------------------------------------------------
    from scipy.special import erf

    w1s = W1[:DX] * WSCALE                      # [128, 256]
    w1_hi = _q8(w1s)
    w1_lo = _q8(w1s - w1_hi.astype(np.float32))
    w2s = W2 * WSCALE                           # [256, 256]
    w2_hi = _q8(w2s)

    # w1: k1 = hi, k2 = lo (same xl streamed through both k-tiles)
    w1q = np.zeros((2, 128, 256), dtype=FP8)
    for h in range(2):
        w1q[h, :, 0:128] = w1_hi[:, h * 128:(h + 1) * 128]
        w1q[h, :, 128:256] = w1_lo[:, h * 128:(h + 1) * 128]
    # w2: ktile t covers contraction rows t*128..t*128+127
    w2hiq = np.zeros((2, 128, 256), dtype=FP8)
    for h in range(2):
        for t in range(2):
            w2hiq[h, :, t * 128:(t + 1) * 128] = \
                w2_hi[t * 128:(t + 1) * 128, h * 128:(h + 1) * 128]

    zb = Z @ W1[DX:DX + DZ] + b1                # [B, 256] exact f32
    zb_blk = np.zeros((nb_tot, H), dtype=np.float32)
    lbl = blk_label >= 0
    zb_blk[lbl] = zb[blk_label[lbl]]
    # zb_d[core]: [128, 2*nblk] with col h*nblk+i = zb of block i, m-half h
    zbd = zb_blk.reshape(N_CORES, nblk, 2, 128).transpose(0, 3, 2, 1)
    zbd = np.ascontiguousarray(zbd).reshape(N_CORES, 128, 2 * nblk)

    # relu2 bias with the per-segment W2-quantization mean-correction:
    # b2c = 64*(b2 - E[h1|seg] @ (W2_hi/64 - W2)), E[h1|seg] from a Gaussian
    # closed form over the actual shipped-data column moments.
    xlf = xl8.astype(np.float32)
    mcol = xlf.mean(axis=0)                     # [128]
    vcol = xlf.var(axis=0)                      # [128]
    w1eff = (w1_hi.astype(np.float32) + w1_lo.astype(np.float32)) / WSCALE
    mu = mcol @ w1eff                           # [256]
    sig = np.sqrt(np.maximum(vcol @ (w1eff ** 2), 1e-12))
    muz = mu[None, :] + zb                      # [B, 256]
    u = muz / sig[None, :]
    Phi = 0.5 * (1.0 + erf(u / np.sqrt(2.0)))
    phi = np.exp(-0.5 * u * u) / np.sqrt(2.0 * np.pi)
    Eh1 = sig[None, :] * phi + muz * Phi        # [B, 256]
    dW2 = w2_hi.astype(np.float32) / WSCALE - W2
    b2c_seg = WSCALE * (b2[None, :] - Eh1 @ dW2)   # [B, 256]
    b2c_blk = np.broadcast_to((WSCALE * b2)[None, :],
                              (nb_tot, H)).copy().astype(np.float32)
    b2c_blk[lbl] = b2c_seg[blk_label[lbl]]
    b2cd = b2c_blk.reshape(N_CORES, nblk, 2, 128).transpose(0, 3, 2, 1)
    b2cd = np.ascontiguousarray(b2cd).reshape(N_CORES, 128, 2 * nblk)

    # ---- run on 8 cores ---------------------------------------------------
    nc = _get_program(nblk)
    in_maps = []
    for c in range(N_CORES):
        in_maps.append({
            "xt": xt[c], "w1": w1q, "w2hi": w2hiq,
            "zb": zbd[c], "b2c": b2cd[c],
        })
    global _last_in_maps
    _last_in_maps = in_maps
    res = run_bass_kernel_spmd(nc, in_maps, list(range(N_CORES)))

    # ---- host epilogue ----------------------------------------------------
    per_core = []
    for c in range(N_CORES):
        o = res.results[c]["out"]               # [128, 2*nblk]
        per_core.append(np.stack([o[:, 0:nblk], o[:, nblk:2 * nblk]], axis=0))
    sums = np.concatenate(per_core, axis=2)     # [2, 128, nb_tot]
    sums = sums.transpose(2, 0, 1).reshape(nb_tot, H)  # [nb_tot, 256]

    # pad-cell contribution: xl = 0 -> h1 = fp8(relu(zb))
    h1p = _q8(np.maximum(zb, 0.0)).astype(np.float32)          # [B, 256]
    pre = h1p @ w2_hi.astype(np.float32) + b2c_seg
    h2p = np.maximum(pre, 0.0)                                 # [B, 256]

    r = blk_real[lbl]
    labs = blk_label[lbl]
    corr = (BLK - r).astype(np.float32)[:, None] * h2p[labs]

    S = np.zeros((nseg, H), dtype=np.float32)
    np.add.at(S, labs, sums[lbl] - corr)
    S /= WSCALE

    denom = np.maximum(counts, 1).astype(np.float32)[:, None]
    Y = S @ W3 / denom + b3[None, :]
    Y[counts == 0] = 0.0
    return Y.astype(np.float32)


# revision 16
# speedup vs baseline: 1.0840x; 1.0840x over previous
"""Trainium2 Bass kernel for CompositionModel (gnn_message_passing).

Model: per-cell MLP over [log1p(X) ++ Z[cell_to_batch]] followed by a
segment-mean over batch labels.

Strategy (v2):
  * Host: log1p(X) precomputed and shipped fp8; cells sorted by segment and
    each segment padded to a multiple of 512 so every 512-cell block is
    single-segment; X blocks shipped twice (side by side) so one fp8
    DoubleRow matmul applies W1x_hi (k-tile 1) and W1x_lo (k-tile 2) -- W1
    is then effectively exact.  Z's contribution (Z @ W1z + b1) is a single
    per-block f32 bias vector applied by the ACT engine during relu1.
  * Device (8 cores, data-parallel over blocks, identical static program):
      L1: 2 fp8 DR matmuls -> PSUM; ACT relu1 (+zb bias, /64 descale) ->
      fp8 h1; L2: per m-half one DR matmul with W2_hi (full block) plus one
      DR matmul with 2*W2_lo on the first half of the columns (first-order
      exact through the segment mean); DVE tensor_scalar does
      bias+relu+cast AND the per-block segment sum via accum_out.
      The third (linear) layer commutes with the segment sum and is applied
      on the host to the 512x256 segment sums instead of 500k cells.
  * Host epilogue: subtract the analytically-known contribution of pad
    cells (xl=0 -> h1 = relu(zb)), combine block sums into segment sums,
    undo the x64 weight scale, apply W3/b3 and divide by true counts.
"""

import numpy as np
import ml_dtypes

import concourse.bacc as bacc
import concourse.mybir as mybir
import concourse.tile as tile
from concourse.bass_utils import run_bass_kernel_spmd

BF16 = ml_dtypes.bfloat16
FP8 = ml_dtypes.float8_e4m3fn

N_CORES = 8
DX = 128
DZ = 32
H = 256
B = 512
BLK = 512          # cells per block == segment pad quantum
SB = 4             # blocks per superblock (one DMA)
NBLK = 132         # blocks per core (fits the fixed reference input)
WSCALE = 64.0      # fp8 pre-scale on W1/W2/b2, divided out at the end

_compiled = {}
_last_in_maps = None


def _build_program(nblk):
    f32 = mybir.dt.float32
    bf16 = mybir.dt.bfloat16
    fp8 = mybir.dt.float8e4
    Alu = mybir.AluOpType
    Act = mybir.ActivationFunctionType
    DR = mybir.MatmulPerfMode.DoubleRow
    nsb = nblk // SB

    nc = bacc.Bacc("TRN2", target_bir_lowering=False, debug=False,
                   num_devices=N_CORES)

    # [super][p][xl0..xl3 | xl0..xl3]  (k-tile stride = SB*BLK)
    xt_d = nc.dram_tensor("xt", [nsb, DX, 2 * SB * BLK], fp8,
                          kind="ExternalInput")
    # [m-half][p, ktile*128] fp8: k1 = 64*W1x_hi, k2 = 64*W1x_lo
    w1_d = nc.dram_tensor("w1", [2, 128, 256], fp8, kind="ExternalInput")
    w2hi_d = nc.dram_tensor("w2hi", [2, 128, 256], fp8, kind="ExternalInput")
    zb_d = nc.dram_tensor("zb", [128, 2 * nblk], f32, kind="ExternalInput")
    # per-(block, m-half) relu2 bias: 64*b2 minus the W2-quantization
    # mean-correction for the block's segment
    b2c_d = nc.dram_tensor("b2c", [128, 2 * nblk], f32, kind="ExternalInput")
    out_d = nc.dram_tensor("out", [128, 2 * nblk], f32, kind="ExternalOutput")

    with tile.TileContext(nc) as tc:
        with tc.tile_pool(name="consts", bufs=1) as cpool, \
             tc.tile_pool(name="xt", bufs=3) as xtpool, \
             tc.tile_pool(name="h1", bufs=3) as h1pool, \
             tc.tile_pool(name="hsc", bufs=2) as hscpool, \
             tc.tile_pool(name="psum", bufs=2, space="PSUM") as psum:

            w1t, w2hit = [], []
            for h in range(2):
                for lst, src in ((w1t, w1_d), (w2hit, w2hi_d)):
                    w = cpool.tile([128, 256], fp8, tag=f"w{len(lst)}_{id(src)}")
                    nc.sync.dma_start(w[:], src[h])
                    lst.append(w[:].rearrange("p (k m) -> p k m", k=2))
            zbt = cpool.tile([128, 2 * nblk], f32, tag="zbt")
            nc.sync.dma_start(zbt[:], zb_d[:])
            b2ct = cpool.tile([128, 2 * nblk], f32, tag="b2ct")
            nc.sync.dma_start(b2ct[:], b2c_d[:])
            zeros = cpool.tile([128, 512], bf16, tag="zeros")
            nc.vector.memset(zeros[:], 0.0)
            outt = cpool.tile([128, 2 * nblk], f32, tag="outt")

            xt_tiles = {}
            ps1_tiles = {}
            h1_tiles = {}
            ps2_tiles = {}

            def load_super(j):
                if j >= nsb:
                    return
                t = xtpool.tile([DX, 2 * SB * BLK], fp8, tag="xt")
                nc.sync.dma_start(t[:], xt_d[j])
                xt_tiles[j] = t

            def emit_l1(i):
                j, b = divmod(i, SB)
                xt = xt_tiles[j]
                xv = xt[:].rearrange("p (k g c) -> p g k c", k=2, g=SB)[:, b]
                ps1 = psum.tile([128, 1024], f32, tag="ps1")
                nc.tensor.matmul(ps1[:, 0:512], w1t[0], xv,
                                 start=True, stop=True, perf_mode=DR)
                nc.tensor.matmul(ps1[:, 512:1024], w1t[1], xv,
                                 start=True, stop=True, perf_mode=DR)
                ps1_tiles[i] = ps1
                if b == SB - 1:
                    xt_tiles.pop(j)

            def emit_relu1(i):
                ps1 = ps1_tiles.pop(i)
                h1 = h1pool.tile([128, 1024], fp8, tag="h1")
                nc.scalar.activation(h1[:, 0:512], ps1[:, 0:512], Act.Relu,
                                     bias=zbt[:, i:i + 1],
                                     scale=1.0 / WSCALE)
                nc.scalar.activation(h1[:, 512:1024], ps1[:, 512:1024],
                                     Act.Relu,
                                     bias=zbt[:, nblk + i:nblk + i + 1],
                                     scale=1.0 / WSCALE)
                h1_tiles[i] = h1

            def emit_l2(i):
                h1 = h1_tiles.pop(i)
                h1v = h1[:].rearrange("p (k c) -> p k c", k=2)
                ps2 = psum.tile([128, 1024], f32, tag="ps2")
                for h in range(2):
                    o = h * 512
                    nc.tensor.matmul(ps2[:, o:o + 512], w2hit[h], h1v,
                                     start=True, stop=True, perf_mode=DR)
                ps2_tiles[i] = ps2

            pending_act = []

            def emit_relu2(i):
                ps2 = ps2_tiles[i]
                done = True
                for h in range(2):
                    c = h * nblk + i
                    # out = max(ps2 + b2c, 0); accum_out = sum(out) over cells
                    # DVE is the saturated engine; give every 11th op to ACT,
                    # but emit those late (flush_act) so they never block the
                    # next relu1 in ACT's strict-FIFO queue
                    if (2 * i + h) % 11 == 10:
                        pending_act.append((i, h))
                        done = False
                    else:
                        hsc = hscpool.tile([128, 512], bf16, tag=f"hsc{h}")
                        nc.vector.scalar_tensor_tensor(
                            hsc[:], ps2[:, h * 512:(h + 1) * 512],
                            b2ct[:, c:c + 1], zeros[:],
                            op0=Alu.add, op1=Alu.max,
                            accum_out=outt[:, c:c + 1])
                if done:
                    ps2_tiles.pop(i)

            def flush_act(upto):
                while pending_act and pending_act[0][0] <= upto:
                    i, h = pending_act.pop(0)
                    ps2 = ps2_tiles.pop(i)
                    c = h * nblk + i
                    hsc = hscpool.tile([128, 512], bf16, tag=f"hscact{h}")
                    nc.scalar.activation(
                        hsc[:], ps2[:, h * 512:(h + 1) * 512], Act.Relu,
                        bias=b2ct[:, c:c + 1], accum_out=outt[:, c:c + 1])

            load_super(0)
            load_super(1)
            emit_l1(0)
            emit_relu1(0)
            for i in range(1, nblk):
                if i % SB == 0:
                    load_super(i // SB + 1)
                emit_l1(i)
                emit_relu1(i)
                flush_act(i - 2)
                emit_l2(i - 1)
                emit_relu2(i - 1)
            emit_l2(nblk - 1)
            emit_relu2(nblk - 1)
            flush_act(nblk)

            nc.sync.dma_start(out_d[:], outt[:])

    nc.compile()
    return nc


def _get_program(nblk):
    if nblk not in _compiled:
        _compiled[nblk] = _build_program(nblk)
    return _compiled[nblk]


def _q8(x):
    return np.asarray(x, dtype=np.float32).astype(FP8)


def kernel(X, Z, W1, b1, W2, b2, W3, b3, cell_to_batch, sample_idx_batch):
    X = np.asarray(X)
    Z = np.asarray(Z, dtype=np.float32)
    W1 = np.asarray(W1, dtype=np.float32)
    b1 = np.asarray(b1, dtype=np.float32)
    W2 = np.asarray(W2, dtype=np.float32)
    b2 = np.asarray(b2, dtype=np.float32)
    W3 = np.asarray(W3, dtype=np.float32)
    b3 = np.asarray(b3, dtype=np.float32)
    c2b = np.asarray(cell_to_batch).astype(np.int64)
    sib = np.asarray(sample_idx_batch).astype(np.int64)

    n = X.shape[0]
    nseg = sib.shape[0]
    seg = sib[c2b]

    # ---- host layout prep -------------------------------------------------
    order = np.argsort(seg, kind="stable")
    seg_sorted = seg[order]
    counts = np.bincount(seg, minlength=nseg).astype(np.int64)
    padded = ((counts + BLK - 1) // BLK) * BLK
    starts = np.concatenate([[0], np.cumsum(padded)])[:nseg]
    total_pad = int(padded.sum())
    nblk = NBLK
    while total_pad > N_CORES * nblk * BLK:  # safety fallback, recompiles
        nblk += SB
    ntot = N_CORES * nblk * BLK
    nb_tot = ntot // BLK
    run_starts = np.concatenate([[0], np.cumsum(counts)])[:nseg]
    ranks = np.arange(n, dtype=np.int64) - run_starts[seg_sorted]
    slots = starts[seg_sorted] + ranks

    xl8 = _q8(np.log1p(np.asarray(X, dtype=np.float32)))
    Xs = np.zeros((ntot, DX), dtype=FP8)
    Xs[slots] = xl8[order]

    # [core, nblk, 128, 512] -> supers [core, nsb, 128, SB*512] doubled
    nsb = nblk // SB
    xtc = Xs.reshape(N_CORES, nsb, SB, BLK, DX).transpose(0, 1, 4, 2, 3)
    xtc = np.ascontiguousarray(xtc).reshape(N_CORES, nsb, DX, SB * BLK)
    xt = np.concatenate([xtc, xtc], axis=3)
    xt = np.ascontiguousarray(xt)

    # per-block segment labels / real-cell counts
    nb_of_seg = (padded // BLK).astype(np.int64)
    blk_label = np.full(nb_tot, -1, dtype=np.int64)
    fill = np.repeat(np.arange(nseg), nb_of_seg)
    blk_label[:fill.shape[0]] = fill
    blk_real = np.zeros(nb_tot, dtype=np.int64)
    blk_real[:fill.shape[0]] = BLK
    last_blk = (starts + padded) // BLK - 1
    blk_real[last_blk] = counts - (nb_of_seg - 1) * BLK

    # ---- weights ----------------------------------------------------------
    from scipy.special import erf

    w1s = W1[:DX] * WSCALE                      # [128, 256]
    w1_hi = _q8(w1s)
    w1_lo = _q8(w1s - w1_hi.astype(np.float32))
    w2s = W2 * WSCALE                           # [256, 256]
    w2_hi = _q8(w2s)

    # w1: k1 = hi, k2 = lo (same xl streamed through both k-tiles)
    w1q = np.zeros((2, 128, 256), dtype=FP8)
    for h in range(2):
        w1q[h, :, 0:128] = w1_hi[:, h * 128:(h + 1) * 128]
        w1q[h, :, 128:256] = w1_lo[:, h * 128:(h + 1) * 128]
    # w2: ktile t covers contraction rows t*128..t*128+127
    w2hiq = np.zeros((2, 128, 256), dtype=FP8)
    for h in range(2):
        for t in range(2):
            w2hiq[h, :, t * 128:(t + 1) * 128] = \
                w2_hi[t * 128:(t + 1) * 128, h * 128:(h + 1) * 128]

    zb = Z @ W1[DX:DX + DZ] + b1                # [B, 256] exact f32
    zb_blk = np.zeros((nb_tot, H), dtype=np.float32)
    lbl = blk_label >= 0
    zb_blk[lbl] = zb[blk_label[lbl]]
    # zb_d[core]: [128, 2*nblk] with col h*nblk+i = zb of block i, m-half h
    zbd = zb_blk.reshape(N_CORES, nblk, 2, 128).transpose(0, 3, 2, 1)
    zbd = np.ascontiguousarray(zbd).reshape(N_CORES, 128, 2 * nblk)

    # relu2 bias with the per-segment W2-quantization mean-correction:
    # b2c = 64*(b2 - E[h1|seg] @ (W2_hi/64 - W2)), E[h1|seg] from a Gaussian
    # closed form over the actual shipped-data column moments.
    xlf = xl8.astype(np.float32)
    mcol = xlf.mean(axis=0)                     # [128]
    vcol = xlf.var(axis=0)                      # [128]
    w1eff = (w1_hi.astype(np.float32) + w1_lo.astype(np.float32)) / WSCALE
    mu = mcol @ w1eff                           # [256]
    sig = np.sqrt(np.maximum(vcol @ (w1eff ** 2), 1e-12))
    muz = mu[None, :] + zb                      # [B, 256]
    u = muz / sig[None, :]
    Phi = 0.5 * (1.0 + erf(u / np.sqrt(2.0)))
    phi = np.exp(-0.5 * u * u) / np.sqrt(2.0 * np.pi)
    Eh1 = sig[None, :] * phi + muz * Phi        # [B, 256]
    dW2 = w2_hi.astype(np.float32) / WSCALE - W2
    b2c_seg = WSCALE * (b2[None, :] - Eh1 @ dW2)   # [B, 256]
    b2c_blk = np.broadcast_to((WSCALE * b2)[None, :],
                              (nb_tot, H)).copy().astype(np.float32)
    b2c_blk[lbl] = b2c_seg[blk_label[lbl]]
    b2cd = b2c_blk.reshape(N_CORES, nblk, 2, 128).transpose(0, 3, 2, 1)
    b2cd = np.ascontiguousarray(b2cd).reshape(N_CORES, 128, 2 * nblk)

    # ---- run on 8 cores ---------------------------------------------------
    nc = _get_program(nblk)
    in_maps = []
    for c in range(N_CORES):
        in_maps.append({
            "xt": xt[c], "w1": w1q, "w2hi": w2hiq,
            "zb": zbd[c], "b2c": b2cd[c],
        })
    global _last_in_maps
    _last_in_maps = in_maps
    res = run_bass_kernel_spmd(nc, in_maps, list(range(N_CORES)))

    # ---- host epilogue ----------------------------------------------------
    per_core = []
    for c in range(N_CORES):
        o = res.results[c]["out"]               # [128, 2*nblk]
        per_core.append(np.stack([o[:, 0:nblk], o[:, nblk:2 * nblk]], axis=0))
    sums = np.concatenate(per_core, axis=2)     # [2, 128, nb_tot]
    sums = sums.transpose(2, 0, 1).reshape(nb_tot, H)  # [nb_tot, 256]

    # pad-cell contribution: xl = 0 -> h1 = fp8(relu(zb))
    h1p = _q8(np.maximum(zb, 0.0)).astype(np.float32)          # [B, 256]
    pre = h1p @ w2_hi.astype(np.float32) + b2c_seg
    h2p = np.maximum(pre, 0.0)                                 # [B, 256]

    r = blk_real[lbl]
    labs = blk_label[lbl]
    corr = (BLK - r).astype(np.float32)[:, None] * h2p[labs]

    S = np.zeros((nseg, H), dtype=np.float32)
    np.add.at(S, labs, sums[lbl] - corr)
    S /= WSCALE

    denom = np.maximum(counts, 1).astype(np.float32)[:, None]
    Y = S @ W3 / denom + b3[None, :]
    Y[counts == 0] = 0.0
    return Y.astype(np.float32)


# revision 20
# speedup vs baseline: 1.2137x; 1.1197x over previous
"""Trainium2 Bass kernel for CompositionModel (gnn_message_passing).

Model: per-cell MLP over [log1p(X) ++ Z[cell_to_batch]] followed by a
segment-mean over batch labels.

Strategy (v4):
  * Host: log1p(X) precomputed and shipped fp8; cells sorted by segment and
    each segment padded to a multiple of 512 so every 512-cell block is
    single-segment.  Near-empty tail blocks (<64 real cells) are dropped
    from the device grid and their cells evaluated exactly on the host.
  * L1: one fp8 DoubleRow matmul per m-half.  k-tile 1 streams xl against
    64*W1x (hi fp8); k-tile 2 is a static zeros tile except two all-ones
    rows whose weight rows carry the per-block segment bias
    zb = Z @ W1z + b1 as an fp8 (hi, lo) pair -- GPSIMD copies the two rows
    into the rotating weight tiles each block.  W1 quantization error is
    mean-corrected through zb using the data's exact column means.
  * ACT: relu1 is a single fused [128,1024] op (scale=1/64) -> fp8 h1.
  * L2: one DoubleRow matmul per m-half with 64*W2 (hi fp8); quantization
    is mean-corrected per segment via the relu2 bias:
    b2c = 64*(b2 - E[h1|seg] @ dW2), E[h1|seg] in Gaussian closed form.
  * relu2 + per-block segment sum: DVE scalar_tensor_tensor with accum_out
    (5 of 6 ops) and ACT activation with accum_out (1 of 6, emitted late to
    avoid FIFO head-of-line blocking).
  * The third (linear) layer commutes with the segment sum and is applied
    on the host to the 512x256 segment sums; pad-cell contributions
    (h1 = fp8(relu(zb))) are subtracted analytically.
"""

import numpy as np
import ml_dtypes

import concourse.bacc as bacc
import concourse.mybir as mybir
import concourse.tile as tile
from concourse.bass_utils import run_bass_kernel_spmd

BF16 = ml_dtypes.bfloat16
FP8 = ml_dtypes.float8_e4m3fn

N_CORES = 8
DX = 128
DZ = 32
H = 256
B = 512
BLK = 512          # cells per block == segment pad quantum
SB = 4             # blocks per superblock (one DMA)
NBLK = 128         # blocks per core (fits the fixed reference input)
WSCALE = 64.0      # fp8 pre-scale on W1/W2/zb/b2, divided out at the end
TAIL = 64          # blocks with fewer real cells go to the host
ACT_ROT = 6        # every 6th relu2 op runs on ACT instead of DVE

_compiled = {}
_last_in_maps = None


def _build_program(nblk):
    f32 = mybir.dt.float32
    bf16 = mybir.dt.bfloat16
    fp8 = mybir.dt.float8e4
    Alu = mybir.AluOpType
    Act = mybir.ActivationFunctionType
    DR = mybir.MatmulPerfMode.DoubleRow
    nsb = nblk // SB

    nc = bacc.Bacc("TRN2", target_bir_lowering=False, debug=False,
                   num_devices=N_CORES)

    xt_d = nc.dram_tensor("xt", [nsb, DX, SB * BLK], fp8,
                          kind="ExternalInput")
    # [m-half][p, ktile*128] fp8: k1 = 64*W1x_hi, k2 = zeros (rows 126/127
    # overwritten per block with the zb hi/lo rows)
    w1_d = nc.dram_tensor("w1", [2, 128, 256], fp8, kind="ExternalInput")
    w2hi_d = nc.dram_tensor("w2hi", [2, 128, 256], fp8, kind="ExternalInput")
    # per-(half, block) zb rows: [hi/lo, (h*nblk+i)*128 + m] fp8 at 64x
    zbq_d = nc.dram_tensor("zbq", [2, 2 * nblk * 128], fp8,
                           kind="ExternalInput")
    b2c_d = nc.dram_tensor("b2c", [128, 2 * nblk], f32, kind="ExternalInput")
    out_d = nc.dram_tensor("out", [128, 2 * nblk], f32, kind="ExternalOutput")

    with tile.TileContext(nc) as tc:
        with tc.tile_pool(name="consts", bufs=1) as cpool, \
             tc.tile_pool(name="h1", bufs=3) as h1pool, \
             tc.tile_pool(name="hsc", bufs=2) as hscpool, \
             tc.tile_pool(name="psum", bufs=2, space="PSUM") as psum:

            # xt tiles: DMA fills cols [0:2048]; cols [2048:4096] are the
            # static k-tile 2 (zeros + two all-ones rows)
            xts = []
            for r in range(3):
                t = cpool.tile([DX, 2 * SB * BLK], fp8, tag=f"xt{r}")
                nc.vector.memset(t[:, SB * BLK:2 * SB * BLK], 0.0)
                nc.vector.memset(t[0:2, SB * BLK:2 * SB * BLK], 1.0)
                xts.append(t)

            w1tiles = []
            for h in range(2):
                row = []
                for r in range(4):
                    w = cpool.tile([128, 256], fp8, tag=f"w1_{h}_{r}")
                    nc.sync.dma_start(w[:], w1_d[h])
                    row.append(w)
                w1tiles.append(row)
            w2hit = []
            for h in range(2):
                w = cpool.tile([128, 256], fp8, tag=f"w2_{h}")
                nc.sync.dma_start(w[:], w2hi_d[h])
                w2hit.append(w[:].rearrange("p (k m) -> p k m", k=2))

            zbsrc = cpool.tile([2, 2 * nblk * 128], fp8, tag="zbsrc")
            nc.sync.dma_start(zbsrc[:], zbq_d[:])
            b2ct = cpool.tile([128, 2 * nblk], f32, tag="b2ct")
            nc.sync.dma_start(b2ct[:], b2c_d[:])
            zeros = cpool.tile([128, 512], bf16, tag="zeros")
            nc.vector.memset(zeros[:], 0.0)
            outt = cpool.tile([128, 2 * nblk], f32, tag="outt")

            ps1_tiles = {}
            h1_tiles = {}
            ps2_tiles = {}

            def load_super(j):
                if j >= nsb:
                    return
                nc.sync.dma_start(xts[j % 3][:, 0:SB * BLK], xt_d[j])

            def emit_zb(i):
                if i >= nblk:
                    return
                r = i % 4
                for h in range(2):
                    o = (h * nblk + i) * 128
                    nc.gpsimd.tensor_copy(
                        w1tiles[h][r][0:2, 128:256],
                        zbsrc[0:2, o:o + 128])

            def emit_l1(i):
                j, b = divmod(i, SB)
                xt = xts[j % 3]
                xv = xt[:].rearrange("p (k g c) -> p g k c", k=2, g=SB)[:, b]
                ps1 = psum.tile([128, 1024], f32, tag="ps1")
                r = i % 4
                for h in range(2):
                    w1v = w1tiles[h][r][:].rearrange("p (k m) -> p k m", k=2)
                    nc.tensor.matmul(ps1[:, h * 512:(h + 1) * 512], w1v, xv,
                                     start=True, stop=True, perf_mode=DR)
                ps1_tiles[i] = ps1

            def emit_relu1(i):
                ps1 = ps1_tiles.pop(i)
                h1 = h1pool.tile([128, 1024], fp8, tag="h1")
                nc.scalar.activation(h1[:], ps1[:], Act.Relu,
                                     scale=1.0 / WSCALE)
                h1_tiles[i] = h1

            def emit_l2(i):
                h1 = h1_tiles.pop(i)
                h1v = h1[:].rearrange("p (k c) -> p k c", k=2)
                ps2 = psum.tile([128, 1024], f32, tag="ps2")
                for h in range(2):
                    o = h * 512
                    nc.tensor.matmul(ps2[:, o:o + 512], w2hit[h], h1v,
                                     start=True, stop=True, perf_mode=DR)
                ps2_tiles[i] = ps2

            pending_act = []

            def emit_relu2(i):
                ps2 = ps2_tiles[i]
                done = True
                for h in range(2):
                    c = h * nblk + i
                    # out = max(ps2 + b2c, 0); accum_out = sum(out) over cells
                    if (2 * i + h) % ACT_ROT == ACT_ROT - 1:
                        pending_act.append((i, h))
                        done = False
                    else:
                        hsc = hscpool.tile([128, 512], bf16, tag=f"hsc{h}")
                        nc.vector.scalar_tensor_tensor(
                            hsc[:], ps2[:, h * 512:(h + 1) * 512],
                            b2ct[:, c:c + 1], zeros[:],
                            op0=Alu.add, op1=Alu.max,
                            accum_out=outt[:, c:c + 1])
                if done:
                    ps2_tiles.pop(i)

            def flush_act(upto):
                while pending_act and pending_act[0][0] <= upto:
                    i, h = pending_act.pop(0)
                    ps2 = ps2_tiles.pop(i)
                    c = h * nblk + i
                    hsc = hscpool.tile([128, 512], bf16, tag=f"hscact{h}")
                    nc.scalar.activation(
                        hsc[:], ps2[:, h * 512:(h + 1) * 512], Act.Relu,
                        bias=b2ct[:, c:c + 1], accum_out=outt[:, c:c + 1])

            load_super(0)
            load_super(1)
            emit_zb(0)
            emit_zb(1)
            emit_zb(2)
            emit_l1(0)
            emit_relu1(0)
            for i in range(1, nblk):
                if i % SB == 0:
                    load_super(i // SB + 1)
                emit_zb(i + 2)
                emit_l1(i)
                emit_relu1(i)
                flush_act(i - 2)
                emit_l2(i - 1)
                emit_relu2(i - 1)
            emit_l2(nblk - 1)
            emit_relu2(nblk - 1)
            flush_act(nblk)

            nc.sync.dma_start(out_d[:], outt[:])

    nc.compile()
    return nc


def _get_program(nblk):
    if nblk not in _compiled:
        _compiled[nblk] = _build_program(nblk)
    return _compiled[nblk]


def _q8(x):
    return np.asarray(x, dtype=np.float32).astype(FP8)


def kernel(X, Z, W1, b1, W2, b2, W3, b3, cell_to_batch, sample_idx_batch):
    from scipy.special import erf

    X = np.asarray(X)
    Z = np.asarray(Z, dtype=np.float32)
    W1 = np.asarray(W1, dtype=np.float32)
    b1 = np.asarray(b1, dtype=np.float32)
    W2 = np.asarray(W2, dtype=np.float32)
    b2 = np.asarray(b2, dtype=np.float32)
    W3 = np.asarray(W3, dtype=np.float32)
    b3 = np.asarray(b3, dtype=np.float32)
    c2b = np.asarray(cell_to_batch).astype(np.int64)
    sib = np.asarray(sample_idx_batch).astype(np.int64)

    n = X.shape[0]
    nseg = sib.shape[0]
    seg = sib[c2b]

    # ---- host layout prep -------------------------------------------------
    order = np.argsort(seg, kind="stable")
    seg_sorted = seg[order]
    counts = np.bincount(seg, minlength=nseg).astype(np.int64)
    nb_of_seg = (counts + BLK - 1) // BLK
    last_real = counts - (nb_of_seg - 1) * BLK
    drop = (last_real < TAIL) & (nb_of_seg >= 2)   # tail blocks -> host
    nb_dev = nb_of_seg - drop
    dev_cap = nb_dev * BLK
    starts = np.concatenate([[0], np.cumsum(dev_cap)])[:nseg]
    total_dev = int(dev_cap.sum())
    nblk = NBLK
    while total_dev > N_CORES * nblk * BLK:  # safety fallback, recompiles
        nblk += SB
    ntot = N_CORES * nblk * BLK
    nb_tot = ntot // BLK

    run_starts = np.concatenate([[0], np.cumsum(counts)])[:nseg]
    ranks = np.arange(n, dtype=np.int64) - run_starts[seg_sorted]
    on_dev = ranks < dev_cap[seg_sorted]
    slots = starts[seg_sorted] + ranks

    xl8 = _q8(np.log1p(np.asarray(X, dtype=np.float32)))
    Xs = np.zeros((ntot, DX), dtype=FP8)
    Xs[slots[on_dev]] = xl8[order[on_dev]]

    nsb = nblk // SB
    xtc = Xs.reshape(N_CORES, nsb, SB, BLK, DX).transpose(0, 1, 4, 2, 3)
    xt = np.ascontiguousarray(xtc).reshape(N_CORES, nsb, DX, SB * BLK)

    # per-block segment labels / real-cell counts
    blk_label = np.full(nb_tot, -1, dtype=np.int64)
    fill = np.repeat(np.arange(nseg), nb_dev)
    blk_label[:fill.shape[0]] = fill
    blk_real = np.zeros(nb_tot, dtype=np.int64)
    blk_real[:fill.shape[0]] = BLK
    last_blk = (starts + dev_cap) // BLK - 1
    keep = ~drop
    blk_real[last_blk[keep]] = counts[keep] - (nb_dev[keep] - 1) * BLK

    # ---- weights ----------------------------------------------------------
    W1x = W1[:DX]
    w1_hi = _q8(W1x * WSCALE)
    w1dev = w1_hi.astype(np.float32) / WSCALE
    dW1 = w1dev - W1x
    w2_hi = _q8(W2 * WSCALE)
    dW2 = w2_hi.astype(np.float32) / WSCALE - W2

    w1q = np.zeros((2, 128, 256), dtype=FP8)
    for h in range(2):
        w1q[h, :, 0:128] = w1_hi[:, h * 128:(h + 1) * 128]
    w2hiq = np.zeros((2, 128, 256), dtype=FP8)
    for h in range(2):
        for t in range(2):
            w2hiq[h, :, t * 128:(t + 1) * 128] = \
                w2_hi[t * 128:(t + 1) * 128, h * 128:(h + 1) * 128]

    # zb with the W1-quantization mean-correction, shipped as fp8 hi+lo
    xlf = xl8.astype(np.float32)
    mcol = xlf.mean(axis=0)
    vcol = xlf.var(axis=0)
    zb = Z @ W1[DX:DX + DZ] + b1 - mcol @ dW1    # [B, 256]
    zbs = zb * WSCALE
    zb_hi = _q8(zbs)
    zb_lo = _q8(zbs - zb_hi.astype(np.float32))
    zbq = (zb_hi.astype(np.float32) + zb_lo.astype(np.float32)) / WSCALE

    lbl = blk_label >= 0
    labs = blk_label[lbl]

    def per_block_rows(src):                     # [B, 256] -> [core, 2*n*128]
        blkv = np.zeros((nb_tot, H), dtype=src.dtype)
        blkv[lbl] = src[blk_label[lbl]]
        a = blkv.reshape(N_CORES, nblk, 2, 128).transpose(0, 2, 1, 3)
        return np.ascontiguousarray(a).reshape(N_CORES, 2 * nblk * 128)

    zbqd = np.stack([per_block_rows(zb_hi), per_block_rows(zb_lo)], axis=1)

    # E[h1|seg] Gaussian closed form -> per-segment W2 mean-correction
    mu = mcol @ w1dev
    sig = np.sqrt(np.maximum(vcol @ (w1dev ** 2), 1e-12))
    muz = mu[None, :] + zbq
    u = muz / sig[None, :]
    Phi = 0.5 * (1.0 + erf(u / np.sqrt(2.0)))
    phi = np.exp(-0.5 * u * u) / np.sqrt(2.0 * np.pi)
    Eh1 = sig[None, :] * phi + muz * Phi
    b2c_seg = WSCALE * (b2[None, :] - Eh1 @ dW2)   # [B, 256]
    b2c_blk = np.broadcast_to((WSCALE * b2)[None, :],
                              (nb_tot, H)).copy().astype(np.float32)
    b2c_blk[lbl] = b2c_seg[blk_label[lbl]]
    b2cd = b2c_blk.reshape(N_CORES, nblk, 2, 128).transpose(0, 3, 2, 1)
    b2cd = np.ascontiguousarray(b2cd).reshape(N_CORES, 128, 2 * nblk)

    # ---- run on 8 cores ---------------------------------------------------
    nc = _get_program(nblk)
    in_maps = []
    for c in range(N_CORES):
        in_maps.append({
            "xt": xt[c], "w1": w1q, "w2hi": w2hiq,
            "zbq": zbqd[c], "b2c": b2cd[c],
        })
    global _last_in_maps
    _last_in_maps = in_maps
    res = run_bass_kernel_spmd(nc, in_maps, list(range(N_CORES)))

    # ---- host epilogue ----------------------------------------------------
    per_core = []
    for c in range(N_CORES):
        o = res.results[c]["out"]               # [128, 2*nblk]
        per_core.append(np.stack([o[:, 0:nblk], o[:, nblk:2 * nblk]], axis=0))
    sums = np.concatenate(per_core, axis=2)     # [2, 128, nb_tot]
    sums = sums.transpose(2, 0, 1).reshape(nb_tot, H)  # [nb_tot, 256]

    # pad-cell contribution: xl = 0 -> h1 = fp8(relu(zbq))
    h1p = _q8(np.maximum(zbq, 0.0)).astype(np.float32)         # [B, 256]
    pre = h1p @ w2_hi.astype(np.float32) + b2c_seg
    h2p = np.maximum(pre, 0.0)                                 # [B, 256]

    r = blk_real[lbl]
    corr = (BLK - r).astype(np.float32)[:, None] * h2p[labs]

    S = np.zeros((nseg, H), dtype=np.float32)
    np.add.at(S, labs, sums[lbl] - corr)
    S /= WSCALE

    # host-evaluated tail cells (exact f32 math)
    idx_host = order[~on_dev]
    if idx_host.shape[0]:
        hh1 = np.maximum(np.log1p(X[idx_host].astype(np.float32)) @ W1x
                         + (Z @ W1[DX:DX + DZ] + b1)[seg[idx_host]], 0.0)
        hh2 = np.maximum(hh1 @ W2 + b2, 0.0)
        np.add.at(S, seg[idx_host], hh2)

    denom = np.maximum(counts, 1).astype(np.float32)[:, None]
    Y = S @ W3 / denom + b3[None, :]
    Y[counts == 0] = 0.0
    return Y.astype(np.float32)
